# revision 1
# baseline (speedup 1.0000x reference)
"""Trainium2 Bass kernel for nn_Attention: 16-head attention layer, B=2, S=2048, H=1024.

Strategy (Megatron-style tensor parallel over heads, 8 cores x 2 heads):
  - Host transposes hidden_states once (XT [H, B*S]) and pre-rounds all matmul
    inputs to fp32r (TF32-like: 11-bit mantissa) so every matmul runs at the
    full 1-cycle/row PE rate with fp32 accumulation.
  - Each core computes its 2 heads' q/k/v via XT @ its W slice (transposed
    layout), attention with softmax folded as exp -> matmul-rowsum -> late
    normalization, then a partial dense projection over its 128 ctx columns.
  - Host sums the 8 partial dense outputs and adds dense_b.

All computed on device except the final 8-way partial reduction (done at
gather time on host, per the Megatron all-reduce-after-dense recipe).
"""
import os
import numpy as np

B, S, H, NH = 2, 2048, 1024, 16
HD = H // NH            # 64
BS = B * S              # 4096
NCORES = 8
ROWS_PER_CORE = 3 * HD * 2   # 384 qkv rows per core
DPC = 2 * HD                 # 128 ctx/dense columns per core

_CACHE = {}


def _round_fp32r(x):
    bits = np.ascontiguousarray(x, dtype=np.float32).view(np.uint32)
    lsb = (bits >> np.uint32(12)) & np.uint32(1)
    return ((bits + np.uint32(0x7FF) + lsb) & np.uint32(0xFFFFF000)).view(np.float32)


def _build_program():
    import concourse.mybir as mybir
    import concourse.tile as tile
    from concourse import bacc

    F32 = mybir.dt.float32
    F32R = mybir.dt.float32r
    Act = mybir.ActivationFunctionType

    nc = bacc.Bacc("TRN2", target_bir_lowering=False, debug=False,
                   num_devices=NCORES)
    xt = nc.dram_tensor("xt", [H, BS], F32R, kind="ExternalInput").ap()
    w1t = nc.dram_tensor("w1t", [H, ROWS_PER_CORE], F32R, kind="ExternalInput").ap()
    b1 = nc.dram_tensor("b1", [128, 3], F32, kind="ExternalInput").ap()
    w2t0 = nc.dram_tensor("w2t0", [HD, H], F32R, kind="ExternalInput").ap()
    w2t1 = nc.dram_tensor("w2t1", [HD, H], F32R, kind="ExternalInput").ap()
    eye2 = nc.dram_tensor("eye2", [128, HD], F32R, kind="ExternalInput").ap()
    ones2 = nc.dram_tensor("ones2", [128, HD], F32R, kind="ExternalInput").ap()
    out = nc.dram_tensor("out", [BS, H], F32, kind="ExternalOutput").ap()

    NK = H // 128          # 8 contraction chunks for qkv
    NN = BS // 512         # 8 token blocks of 512
    NQB = S // 512         # 4 query blocks per batch
    NKC = S // 128         # 16 key chunks per batch

    with tile.TileContext(nc) as tc, nc.allow_low_precision(reason="fp32r"):
        from contextlib import ExitStack
        with ExitStack() as ctx:
            consts = ctx.enter_context(tc.tile_pool(name="consts", bufs=1))
            mixed = ctx.enter_context(tc.tile_pool(name="mixed", bufs=1))
            ctxp = ctx.enter_context(tc.tile_pool(name="ctxp", bufs=1))
            xtp = ctx.enter_context(tc.tile_pool(name="xtp", bufs=5))
            vsb = ctx.enter_context(tc.tile_pool(name="vsb", bufs=2))
            expp = ctx.enter_context(tc.tile_pool(name="expp", bufs=9))
            sums = ctx.enter_context(tc.tile_pool(name="sums", bufs=2))
            ctxf_p = ctx.enter_context(tc.tile_pool(name="ctxf", bufs=2))
            rbp = ctx.enter_context(tc.tile_pool(name="rbp", bufs=2))
            outs = ctx.enter_context(tc.tile_pool(name="outs", bufs=4))
            ps_sc = ctx.enter_context(tc.tile_pool(name="ps_sc", bufs=2, space="PSUM"))
            ps_ac = ctx.enter_context(tc.tile_pool(name="ps_ac", bufs=2, space="PSUM"))
            ps_ms = ctx.enter_context(tc.tile_pool(name="ps_ms", bufs=2, space="PSUM"))

            # ---- constants ----
            w1big = consts.tile([128, NK, ROWS_PER_CORE], F32R, name="w1big")
            w1r = w1t.rearrange("(k p) r -> p k r", p=128)
            nc.sync.dma_start(w1big[:, 0:1, :], w1r[:, 0:1, :])
            nc.sync.dma_start(w1big[:, 1:NK // 2, :], w1r[:, 1:NK // 2, :])
            nc.sync.dma_start(w1big[:, NK // 2:NK, :], w1r[:, NK // 2:NK, :])
            b1sb = consts.tile([128, 3], F32, name="b1")
            nc.sync.dma_start(b1sb[:], b1)
            warm = consts.tile([1, 1], F32, name="warm")
            nc.scalar.activation(warm[0:1, 0:1], b1sb[0:1, 0:1], Act.Exp)
            eye2sb = consts.tile([128, HD], F32R, name="eye2")
            nc.sync.dma_start(eye2sb[:], eye2)
            ones2sb = consts.tile([128, HD], F32R, name="ones2")
            nc.sync.dma_start(ones2sb[:], ones2)
            w2sb = consts.tile([128, H], F32R, name="w2pack")
            nc.sync.dma_start(w2sb[0:HD, :], w2t0)
            nc.sync.dma_start(w2sb[HD:128, :], w2t1)

            # ---- phase A building blocks ----
            qt = mixed.tile([128, BS], F32R, name="qt")
            kt = mixed.tile([128, BS], F32R, name="kt")
            vt = mixed.tile([128, BS], F32R, name="vt")
            mix_dst = [qt, kt, vt]
            KG = 4  # k-chunks per xt DMA

            def emit_qkv_nblock(n, fine=False):
                """mixedT[:, n*512:(n+1)*512] = W1 @ XT block (+bias).
                m-outer / k-inner: one PSUM slot at a time, PE K-contiguous.
                fine=True splits the loads per k-chunk so the first matmul
                starts as soon as 256KB has landed (kernel warmup)."""
                xts = []
                for kg in range(NK // KG):
                    xt_t = xtp.tile([128, KG, 512], F32R, name="xt")
                    if fine:
                        for c in range(KG):
                            k = kg * KG + c
                            nc.sync.dma_start(
                                xt_t[:, c, :],
                                xt[k * 128:(k + 1) * 128,
                                   n * 512:(n + 1) * 512])
                    else:
                        nc.sync.dma_start(
                            xt_t[:],
                            xt[kg * KG * 128:(kg + 1) * KG * 128,
                               n * 512:(n + 1) * 512].rearrange(
                                   "(c p) f -> p c f", p=128))
                    xts.append(xt_t)
                for m in range(3):
                    ps = ps_ac.tile([128, 512], F32, name=f"qkv{m}", tag="acc")
                    for k in range(NK):
                        nc.tensor.matmul(
                            ps[:],
                            w1big[:, k, m * 128:(m + 1) * 128],
                            xts[k // KG][:, k % KG, :],
                            start=(k == 0), stop=(k == NK - 1))
                    nc.scalar.activation(
                        mix_dst[m][:, n * 512:(n + 1) * 512], ps[:],
                        Act.Identity, bias=b1sb[:, m:m + 1])

            def emit_vprep(b):
                vbig = {}
                for j in range(2):
                    vb = vsb.tile([128, NKC * (HD + 1)], F32R, name=f"vbig{j}")
                    ones_view = vb[:].rearrange(
                        "p (c w) -> p c w", w=HD + 1)[:, :, HD:HD + 1]
                    nc.vector.tensor_copy(ones_view, ones2sb[:, 0:NKC])
                    for kc in range(NKC):
                        pt = ps_ms.tile([128, HD], F32R, name="vtr", tag="misc")
                        nc.tensor.transpose(
                            pt[:],
                            vt[64 * j:64 * j + 64,
                               b * S + kc * 128:b * S + (kc + 1) * 128],
                            eye2sb[64 * j:64 * j + 64, :])
                        nc.vector.tensor_copy(
                            vb[:, kc * (HD + 1):kc * (HD + 1) + HD], pt[:])
                    vbig[j] = vb
                return vbig

            def emit_attention_kc(b, qb, vbig):
                ctxps = {j: ps_ac.tile([HD + 1, 512], F32, name=f"ctxps{j}",
                                       tag="acc")
                         for j in range(2)}
                for kc in range(NKC):
                    sp2 = ps_sc.tile([128, 1024], F32, name="scores")
                    for j in range(2):
                        nc.tensor.matmul(
                            sp2[:, j * 512:(j + 1) * 512],
                            kt[64 * j:64 * j + 64,
                               b * S + kc * 128:b * S + (kc + 1) * 128],
                            qt[64 * j:64 * j + 64,
                               b * S + qb * 512:b * S + (qb + 1) * 512],
                            start=True, stop=True)
                    et2 = expp.tile([128, 1024], F32R, name="exp")
                    nc.scalar.activation(et2[:], sp2[:], Act.Exp, scale=0.125)
                    for j in range(2):
                        nc.tensor.matmul(
                            ctxps[j][:],
                            vbig[j][:, kc * (HD + 1):(kc + 1) * (HD + 1)],
                            et2[:, j * 512:(j + 1) * 512],
                            start=(kc == 0), stop=(kc == NKC - 1))
                return ctxps

            def emit_norm(b, qb, ctxps, cts):
                for j in range(2):
                    # free the accumulator bank after a single copy; the rest
                    # of the normalization runs from SBUF off the critical path
                    ctxf = ctxf_p.tile([HD + 1, 512], F32, name="ctxf")
                    nc.vector.tensor_copy(ctxf[:], ctxps[j][:])
                    ss = sums.tile([1, 512], F32, name="sums")
                    nc.vector.tensor_copy(ss[0:1, :], ctxf[HD:HD + 1, :])
                    rbb = rbp.tile([HD, 512], F32, name="rbb")
                    nc.gpsimd.partition_broadcast(rbb[:], ss[0:1, :])
                    rb = rbp.tile([HD, 512], F32, name="rb")
                    nc.vector.reciprocal_approx_fast(rb[:], rbb[:])
                    nc.vector.tensor_mul(
                        cts[64 * j:64 * (j + 1), qb * 512:(qb + 1) * 512],
                        ctxf[0:HD, :], rb[:])

            def emit_dense_qb(b, qb, cts):
                """Dense partial for the 512-token block qb (4 t-chunks)."""
                for t4 in range(4):
                    t = qb * 4 + t4
                    ob = outs.tile([128, H], F32, name="ostage")
                    for nb in range(2):
                        dp = ps_ms.tile([128, 512], F32, name="dense",
                                        tag="misc")
                        nc.tensor.matmul(
                            dp[:], cts[:, t * 128:(t + 1) * 128],
                            w2sb[:, nb * 512:(nb + 1) * 512],
                            start=True, stop=True)
                        nc.vector.tensor_copy(
                            ob[:, nb * 512:(nb + 1) * 512], dp[:])
                    row0 = b * S + t * 128
                    nc.sync.dma_start(out[row0:row0 + 128, :], ob[:])

            # ---- emission schedule ----
            # Serial qkv projection (DMA-paced; PE slack absorbs the v
            # transposes), then per-batch attention with dense interleaved
            # per query block so the epilogue never piles up at the end.
            for n in range(NN // 2):
                emit_qkv_nblock(n, fine=(n == 0))
            vbigs = {0: emit_vprep(0)}
            cts = {0: ctxp.tile([128, S], F32R, name="ctx_0")}
            pend = (0, 0, emit_attention_kc(0, 0, vbigs[0]))
            for n in range(NN // 2, NN):
                emit_qkv_nblock(n)
            cts[1] = ctxp.tile([128, S], F32R, name="ctx_1")
            for b, qb in [(0, 1), (0, 2), (0, 3),
                          (1, 0), (1, 1), (1, 2), (1, 3)]:
                if (b, qb) == (0, 2):
                    # batch-1 v transposes ride the attention window's spare
                    # PE/misc capacity instead of extending phase A
                    vbigs[1] = emit_vprep(1)
                cur = (b, qb, emit_attention_kc(b, qb, vbigs[b]))
                pb, pq, pctx = pend
                emit_norm(pb, pq, pctx, cts[pb])
                emit_dense_qb(pb, pq, cts[pb])
                pend = cur
            pb, pq, pctx = pend
            emit_norm(pb, pq, pctx, cts[pb])
            emit_dense_qb(pb, pq, cts[pb])
    nc.compile()
    return nc


def _prepare_inputs(hidden_states, qkv_w, qkv_b, dense_w):
    """Build per-core input maps (all host-side slicing/transposition)."""
    x = np.ascontiguousarray(hidden_states, dtype=np.float32).reshape(BS, H)
    xt = _round_fp32r(np.ascontiguousarray(x.T))
    eye2 = np.concatenate([np.eye(HD, dtype=np.float32)] * 2, axis=0)
    ones2 = np.ones((128, HD), dtype=np.float32)
    in_maps = []
    for c in range(NCORES):
        base = c * ROWS_PER_CORE
        # per-head row groups within this core's 384 rows: h0 {q,k,v}, h1 {q,k,v}
        rows = {}
        for m in range(3):  # 0=q 1=k 2=v
            rows[m] = np.r_[base + m * HD:base + (m + 1) * HD,
                            base + 192 + m * HD:base + 192 + (m + 1) * HD]
        perm = np.concatenate([rows[0], rows[1], rows[2]])
        w1t = _round_fp32r(np.ascontiguousarray(qkv_w[perm, :].T))   # [H, 384]
        b1 = np.ascontiguousarray(
            np.stack([qkv_b[rows[m]] for m in range(3)], axis=1),
            dtype=np.float32)                                        # [128, 3]
        w2t0 = _round_fp32r(np.ascontiguousarray(
            dense_w[:, c * DPC:c * DPC + HD].T))                     # [64, 1024]
        w2t1 = _round_fp32r(np.ascontiguousarray(
            dense_w[:, c * DPC + HD:(c + 1) * DPC].T))
        in_maps.append({
            "xt": xt, "w1t": w1t, "b1": b1,
            "w2t0": w2t0, "w2t1": w2t1,
            "eye2": eye2, "ones2": ones2,
        })
    return in_maps


def _reference_numpy(hidden_states, attention_mask, qkv_w, qkv_b, dense_w, dense_b):
    """Exact fallback for non-all-ones masks (never hit with spec inputs)."""
    x = np.asarray(hidden_states, dtype=np.float64)
    mask = np.asarray(attention_mask, dtype=np.float64)
    mixed = x @ np.asarray(qkv_w, np.float64).T + np.asarray(qkv_b, np.float64)
    mixed = mixed.reshape(B, S, NH, 3 * HD).transpose(0, 2, 1, 3)
    q, k, v = np.split(mixed, 3, axis=-1)
    scores = np.einsum("bhqd,bhkd->bhqk", q, k) / np.sqrt(HD)
    scores = scores * mask - 10000.0 * (1.0 - mask)
    scores -= scores.max(axis=-1, keepdims=True)
    probs = np.exp(scores)
    probs /= probs.sum(axis=-1, keepdims=True)
    cx = np.einsum("bhqk,bhkd->bhqd", probs, v)
    cx = cx.transpose(0, 2, 1, 3).reshape(B, S, H)
    o = cx @ np.asarray(dense_w, np.float64).T + np.asarray(dense_b, np.float64)
    return o.astype(np.float32)


def _run(inputs, trace=False):
    from concourse.bass_utils import run_bass_kernel_spmd
    if "nc" not in _CACHE:
        _CACHE["nc"] = _build_program()
    nc = _CACHE["nc"]
    in_maps = _prepare_inputs(inputs["hidden_states"], inputs["qkv_w"],
                              inputs["qkv_b"], inputs["dense_w"])
    res = run_bass_kernel_spmd(nc, in_maps, core_ids=list(range(NCORES)),
                               trace=trace)
    partials = np.stack([r["out"] for r in res.results], axis=0)
    full = partials.sum(axis=0, dtype=np.float64)
    full += np.asarray(inputs["dense_b"], dtype=np.float64)
    return full.astype(np.float32).reshape(B, S, H), res


def kernel(hidden_states, attention_mask, qkv_w, qkv_b, dense_w, dense_b):
    hidden_states = np.asarray(hidden_states)
    attention_mask = np.asarray(attention_mask)
    qkv_w = np.asarray(qkv_w)
    qkv_b = np.asarray(qkv_b)
    dense_w = np.asarray(dense_w)
    dense_b = np.asarray(dense_b)
    if not np.all(attention_mask == 1.0):
        return _reference_numpy(hidden_states, attention_mask, qkv_w, qkv_b,
                                dense_w, dense_b)
    out, _ = _run({
        "hidden_states": hidden_states, "qkv_w": qkv_w, "qkv_b": qkv_b,
        "dense_w": dense_w, "dense_b": dense_b,
    }, trace=bool(int(os.environ.get("KERNEL_TRACE", "0"))))
    return out



# revision 50
# speedup vs baseline: 1.1747x; 1.1747x over previous
"""Trainium2 Bass kernel for nn_Attention: 16-head attention, B=2, S=2048, H=1024.

Strategy (Megatron tensor-parallel over heads, 8 cores x 2 heads), v2:
  - All device data in bf16 (fp32 PSUM accumulation), halving HBM traffic.
  - Transposed-context formulation: probabilities are the *stationary* matmul
    operand and V the moving one, so each context matmul streams only 65
    columns (64 dims + a ones column for the softmax denominator) instead of
    512 queries. Stationary loads are free on the PE, halving context cost.
  - V is produced directly in [token, dim] layout by per-token-chunk matmuls
    (x chunk stationary), eliminating the separate V transposes.
  - Normalization is a per-partition scalar multiply (denominator lands in
    the same partition as its query), then a single PE transpose per 128
    tokens feeds the dense projection.
  - The kc loop is software-pipelined (scores run one chunk ahead of the
    exp->context consumers) so the Activation engine never idles; psum->sbuf
    staging runs on DVE/Pool, keeping Act exclusively on exp.
  - Host sums the 8 partial dense outputs (Megatron all-reduce-after-dense),
    adds dense_b and the folded V-bias term.
"""
import os
import numpy as np
import ml_dtypes

B, S, H, NH = 2, 2048, 1024, 16
HD = H // NH            # 64
BS = B * S              # 4096
NCORES = 8
NK = H // 128           # 8 contraction chunks
NQB = S // 512          # 4 query windows per batch
NKC = S // 128          # 16 key chunks per batch

_CACHE = {}
_PHASE_LOG = []   # (label, next-instruction-id) markers, for profiling


def _bf16(x):
    return np.ascontiguousarray(x, dtype=np.float32).astype(ml_dtypes.bfloat16)


def _build_program():
    import concourse.mybir as mybir
    import concourse.tile as tile
    from concourse import bacc
    from contextlib import ExitStack

    F32 = mybir.dt.float32
    BF16 = mybir.dt.bfloat16
    Act = mybir.ActivationFunctionType

    nc = bacc.Bacc("TRN2", target_bir_lowering=False, debug=False,
                   num_devices=NCORES)
    xt = nc.dram_tensor("xt", [H, BS], BF16, kind="ExternalInput").ap()
    w1qk = nc.dram_tensor("w1qk", [H, 256], BF16, kind="ExternalInput").ap()
    # packed consts: crest1 = w1v (8x128 chunks); crest2 = eye | w2t
    crest1 = nc.dram_tensor("crest1", [128, 1024], BF16,
                            kind="ExternalInput").ap()
    crest2 = nc.dram_tensor("crest2", [128, 1152], BF16,
                            kind="ExternalInput").ap()
    qkb = nc.dram_tensor("qkb", [128, 2], F32, kind="ExternalInput").ap()
    out = nc.dram_tensor("out", [BS, H], BF16, kind="ExternalOutput").ap()
    debug = bool(int(os.environ.get("KERNEL_DEBUG", "0")))
    if debug:
        dbg_qt = nc.dram_tensor("dbg_qt", [128, BS], BF16,
                                kind="ExternalOutput").ap()
        dbg_kt = nc.dram_tensor("dbg_kt", [128, BS], BF16,
                                kind="ExternalOutput").ap()
        dbg_vsb = nc.dram_tensor("dbg_vsb", [128, 2 * 2 * NKC * 65], BF16,
                                 kind="ExternalOutput").ap()
        dbg_et = nc.dram_tensor("dbg_et", [128, 1024], BF16,
                                kind="ExternalOutput").ap()
        dbg_cxs = nc.dram_tensor("dbg_cxs", [128, 1024], F32,
                                 kind="ExternalOutput").ap()
        dbg_cts = nc.dram_tensor("dbg_cts", [128, 512], BF16,
                                 kind="ExternalOutput").ap()

    def _mark(label):
        _PHASE_LOG.append(
            (label, int(nc.get_next_instruction_name().split("-")[1])))

    with tile.TileContext(nc) as tc, nc.allow_low_precision(reason="bf16"):
        with ExitStack() as ctx:
            consts = ctx.enter_context(tc.tile_pool(name="consts", bufs=1))
            qkp = ctx.enter_context(tc.tile_pool(name="qkp", bufs=1))
            xtp = ctx.enter_context(tc.tile_pool(name="xtp", bufs=6))
            vsbp = ctx.enter_context(tc.tile_pool(name="vsbp", bufs=34))
            expp = ctx.enter_context(tc.tile_pool(name="expp", bufs=10))
            rsbp = ctx.enter_context(tc.tile_pool(name="rsbp", bufs=2))
            cxp = ctx.enter_context(tc.tile_pool(name="cxp", bufs=3))
            cnp = ctx.enter_context(tc.tile_pool(name="cnp", bufs=3))
            ctsp = ctx.enter_context(tc.tile_pool(name="ctsp", bufs=2))
            obp = ctx.enter_context(tc.tile_pool(name="obp", bufs=4))
            ps_sc = ctx.enter_context(tc.tile_pool(name="ps_sc", bufs=2,
                                                   space="PSUM"))
            ps_ctx = ctx.enter_context(tc.tile_pool(name="ps_ctx", bufs=2,
                                                    space="PSUM"))
            ps_ms = ctx.enter_context(tc.tile_pool(name="ps_ms", bufs=2,
                                                   space="PSUM"))

            # ---- constants ----
            w1qk_sb = consts.tile([128, NK, 256], BF16, name="w1qk")
            w1qk_r = w1qk.rearrange("(k p) m -> p k m", p=128)
            nc.sync.dma_start(w1qk_sb[:, 0:NK // 2, :], w1qk_r[:, 0:NK // 2, :])
            nc.sync.dma_start(w1qk_sb[:, NK // 2:NK, :], w1qk_r[:, NK // 2:NK, :])
            qkb_sb = consts.tile([128, 2], F32, name="qkb")
            nc.sync.dma_start(qkb_sb[:], qkb)
            warm = consts.tile([1, 1], F32, name="warm")
            nc.scalar.activation(warm[0:1, 0:1], qkb_sb[0:1, 0:1], Act.Exp)
            crest1_sb = consts.tile([128, 1024], BF16, name="crest1")
            crest2_sb = consts.tile([128, 1152], BF16, name="crest2")
            w1v_sb = crest1_sb[:, 0:1024].rearrange("p (k m) -> p k m", k=NK)
            eye_sb = crest2_sb[:, 0:128]
            w2_sb = crest2_sb[:, 128:1152]

            qt = qkp.tile([128, BS], BF16, name="qt")
            kt = qkp.tile([128, BS], BF16, name="kt")
            ones_sb = consts.tile([128, 1], BF16, name="ones")
            nc.vector.memset(ones_sb[:], 1.0)
            vsb = {}

            # ---- building blocks ----
            # Every PSUM tile's lifetime (alloc -> matmuls -> drain copy) is
            # emitted contiguously so the misc psum ring can never deadlock
            # on out-of-order buffer reuse.
            def emit_xt_dma(n, halves=False):
                _mark(f"xtdma{n}")
                xt_t = xtp.tile([128, NK, 512], BF16, name="xt")
                src = xt[:, n * 512:(n + 1) * 512].rearrange(
                    "(c p) f -> p c f", p=128)
                if halves:   # split by token half, matching the qk units
                    nc.sync.dma_start(xt_t[:, :, 0:256], src[:, :, 0:256])
                    nc.sync.dma_start(xt_t[:, :, 256:512], src[:, :, 256:512])
                else:
                    nc.sync.dma_start(xt_t[:], src)
                return xt_t

            def _ms_tile(shape, rr=False):
                return ps_ms.tile(shape, F32, name="psqk", tag="misc")

            def emit_qk_unit(n, m, xt_t, rr=False):
                """One projection unit: m=0 -> q rows, m=1 -> k rows.
                Two 256-token halves keep misc psum tiles at 1KB."""
                _mark(f"qk{n}.{'qk'[m]}")
                dst = qt if m == 0 else kt
                for h in (0, 1):
                    ps = _ms_tile([128, 256], rr)
                    for k in range(NK):
                        nc.tensor.matmul(ps[:],
                                         w1qk_sb[:, k, m * 128:(m + 1) * 128],
                                         xt_t[:, k, h * 256:(h + 1) * 256],
                                         start=(k == 0), stop=(k == NK - 1))
                    sl = slice(n * 512 + h * 256, n * 512 + (h + 1) * 256)
                    nc.vector.tensor_scalar_add(dst[:, sl], ps[:],
                                                qkb_sb[:, m:m + 1])

            def alloc_vsb(b):
                pass

            def emit_vT(xt_t, c4, t, rr=False):
                """V (+ones col) for global 128-token chunk t, both heads."""
                _mark(f"vT{t}")
                b, kc = divmod(t, NKC)
                ps = _ms_tile([128, 128], rr)
                for k in range(NK):
                    nc.tensor.matmul(ps[:],
                                     xt_t[:, k, c4 * 128:(c4 + 1) * 128],
                                     w1v_sb[:, k, :],
                                     start=(k == 0), stop=(k == NK - 1))
                vt_sb = vsbp.tile([128, 128], BF16, name="vsb")
                nc.vector.tensor_copy(vt_sb[:], ps[:])
                vsb[(b, kc)] = vt_sb

            def emit_scores_exp(b, qb, kc):
                _mark(f"se.{b}{qb}.{kc}")
                sp = ps_sc.tile([128, 1024], F32, name="sc", tag="sc")
                for j in (0, 1):
                    nc.tensor.matmul(
                        sp[:, j * 512:(j + 1) * 512],
                        kt[64 * j:64 * j + 64,
                           b * S + kc * 128:b * S + (kc + 1) * 128],
                        qt[64 * j:64 * j + 64,
                           b * S + qb * 512:b * S + (qb + 1) * 512],
                        start=True, stop=True)
                et = expp.tile([128, 1024], BF16, name="exp")
                nc.scalar.activation(et[:], sp[:], Act.Exp, scale=0.125)
                return et

            def emit_ctx(b, kc, et, ctxps):
                # start zeroes the whole psum bank, so only the first matmul
                # into each tile starts and only the last stops — the four
                # 65-col query groups all live in that one bank-group.
                _mark(f"cx.{b}.{kc}")
                for j in (0, 1):
                    for qc in range(4):
                        st = et[:, j * 512 + qc * 128:j * 512 + (qc + 1) * 128]
                        nc.tensor.matmul(
                            ctxps[j][:, qc * 128:qc * 128 + 64],
                            st,
                            vsb[(b, kc)][:, j * 64:(j + 1) * 64],
                            start=(kc == 0 and qc == 0),
                            stop=False, skip_group_check=True)
                        nc.tensor.matmul(
                            ctxps[j][:, qc * 128 + 64:qc * 128 + 65],
                            st, ones_sb[:],
                            start=False,
                            stop=(kc == NKC - 1 and qc == 3),
                            skip_group_check=True)

            def emit_norm(ctxps, dbg=False):
                """Drain ctx psum to sbuf fast (frees the accumulation ring
                for the next window), then normalize by the denominator
                column and transpose to [dims, tokens] off the critical
                path; returns the cts tile for the dense stage."""
                _mark("norm")
                cxs = {}
                for j in (0, 1):
                    cxs[j] = cxp.tile([128, 512], F32, name="cxs")
                    nc.vector.tensor_copy(cxs[j][:], ctxps[j][:])
                if dbg:
                    for j in (0, 1):
                        nc.sync.dma_start(dbg_cxs[:, j * 512:(j + 1) * 512],
                                          cxs[j][:])
                rt = rsbp.tile([128, 8, 1], F32, name="recip")
                for j in (0, 1):
                    dview = cxs[j][:].rearrange(
                        "p (g w) -> p g w", w=128)[:, :, 64:65]
                    nc.vector.reciprocal(rt[:, j * 4:(j + 1) * 4, :], dview)
                ctsw = ctsp.tile([128, 512], BF16, name="cts")
                for qc in range(4):
                    cn = cnp.tile([128, 128], BF16, name="cn")
                    for j in (0, 1):
                        nc.vector.tensor_scalar_mul(
                            cn[:, j * 64:(j + 1) * 64],
                            cxs[j][:, qc * 128:qc * 128 + 64],
                            rt[:, j * 4 + qc:j * 4 + qc + 1, 0])
                    pt = ps_ms.tile([128, 128], BF16, name="ctT", tag="misc")
                    nc.tensor.transpose(pt[:], cn[:], eye_sb)
                    nc.vector.tensor_copy(ctsw[:, qc * 128:(qc + 1) * 128],
                                          pt[:])
                if dbg:
                    nc.sync.dma_start(dbg_cts[:], ctsw[:])
                return ctsw

            def emit_dense(b, qb, qc, ctsw, pool=None, tag="misc",
                           split_dma=False, engs=None):
                _mark(f"dn.{b}{qb}.{qc}")
                pool = pool or ps_ms
                ob = obp.tile([128, H], BF16, name="ob")
                row0 = b * S + (qb * 4 + qc) * 128
                for nb in (0, 1):
                    dp = pool.tile([128, 512], F32, name="dp", tag=tag)
                    nc.tensor.matmul(dp[:], ctsw[:, qc * 128:(qc + 1) * 128],
                                     w2_sb[:, nb * 512:(nb + 1) * 512],
                                     start=True, stop=True)
                    sl = slice(nb * 512, (nb + 1) * 512)
                    if engs is not None and engs[(2 * qc + nb) % len(engs)] \
                            is nc.scalar:
                        nc.scalar.copy(ob[:, sl], dp[:])
                    else:
                        nc.vector.tensor_copy(ob[:, sl], dp[:])
                    if split_dma:
                        nc.sync.dma_start(
                            out[row0:row0 + 128, nb * 512:(nb + 1) * 512],
                            ob[:, nb * 512:(nb + 1) * 512])
                if not split_dma:
                    nc.sync.dma_start(out[row0:row0 + 128, :], ob[:])

            # ---- emission schedule ----
            # Per-window kc loop is pipelined one chunk ahead: slot kc emits
            # scores/exp(kc) then ctx(kc-1), so the PE always has the next
            # scores ready before Act finishes the current exp. qkv blocks
            # and vT chunks ride the PE slack inside the windows; the
            # previous window's norm runs at slot 0 and its dense chunks at
            # slots 1/5/9/13.
            # Prologue: only block-0 projection precedes window 0 —
            # everything else rides window slots so the scores/exp chain
            # (which paces the whole kernel) starts as early as possible.
            xts = {}
            xts[0] = emit_xt_dma(0, halves=True)
            nc.sync.dma_start(crest1_sb[:], crest1)
            xts[1] = emit_xt_dma(1)
            emit_qk_unit(0, 1, xts[0])   # k first: scores gate on kt
            emit_qk_unit(0, 0, xts[0])

            windows = [(b, qb) for b in range(B) for qb in range(NQB)]
            # window -> {slot: [hook, ...]}; hooks: ("dma", n) xt load,
            # ("crest2", 0), ("k"/"q", n) projection unit, ("vt", t) chunk.
            def _vt(t):
                return ("vt", t)

            hooks = {
                0: {0: [_vt(0)], 1: [("dma", 2), ("k", 1), _vt(1)],
                    2: [("q", 1), _vt(2)], 3: [_vt(3)], 4: [_vt(4)],
                    5: [("dma", 3), ("crest2", 0), _vt(5)],
                    6: [("k", 2), _vt(6)], 7: [_vt(7)],
                    8: [("q", 2), _vt(8)], 9: [_vt(9)],
                    10: [("k", 3), _vt(10)], 11: [_vt(11)],
                    12: [("q", 3), _vt(12)], 13: [_vt(13)], 14: [_vt(14)],
                    15: [_vt(15)]},
                1: {1: [("dma", 4)], 2: [_vt(16)], 4: [("k", 4)],
                    6: [_vt(17)], 8: [_vt(18)], 10: [_vt(19)]},
                2: {1: [("dma", 5)], 3: [_vt(20)], 4: [("k", 5)],
                    6: [_vt(21)], 8: [("q", 4)], 10: [_vt(22)],
                    12: [_vt(23)]},
                3: {1: [("dma", 6)], 3: [_vt(24)], 4: [("k", 6)],
                    6: [_vt(25)], 8: [_vt(26)], 10: [_vt(27)]},
                4: {1: [("dma", 7)], 3: [("k", 7)], 5: [_vt(28)],
                    6: [("q", 5)], 7: [_vt(29)], 9: [_vt(30)],
                    11: [_vt(31)]},
                5: {4: [("q", 6)], 8: [("q", 7)]},
            }

            pend = None    # (b, qb, ctxps) awaiting norm+dense
            dense_q = []   # deferred dense chunks [(b, qb, qc, ctsw)]
            for w, (b, qb) in enumerate(windows):
                ctxps = {j: ps_ctx.tile([128, 512], F32,
                                        name=f"ctx{j}", tag="acc")
                         for j in (0, 1)}
                et_prev = None
                for kc in range(NKC):
                    et = emit_scores_exp(b, qb, kc)
                    if debug and w == 0 and kc == 0:
                        nc.sync.dma_start(dbg_et[:], et[:])
                    if kc == 0 and pend is not None:
                        pb, pq, pctx = pend
                        ctsw = emit_norm(pctx, dbg=(debug and w == 1))
                        dense_q = [(pb, pq, qc, ctsw) for qc in range(4)]
                        pend = None
                    if kc > 0:
                        emit_ctx(b, kc - 1, et_prev, ctxps)
                    if kc in (2, 6, 10, 14) and dense_q:
                        emit_dense(*dense_q.pop(0))
                    for hk in hooks.get(w, {}).get(kc, []):
                        what, n = hk
                        if what == "dma":
                            xts[n] = emit_xt_dma(n)
                        elif what == "crest2":
                            nc.sync.dma_start(crest2_sb[:], crest2)
                        elif what == "k":
                            emit_qk_unit(n, 1, xts[n])
                        elif what == "q":
                            emit_qk_unit(n, 0, xts[n])
                        else:
                            emit_vT(xts[n // 4], n % 4, n)
                    et_prev = et
                emit_ctx(b, NKC - 1, et_prev, ctxps)
                while dense_q:
                    emit_dense(*dense_q.pop(0))
                pend = (b, qb, ctxps)
            if debug:
                nc.sync.dma_start(dbg_qt[:], qt[:])
                nc.sync.dma_start(dbg_kt[:], kt[:])
                for kc_ in range(NKC):
                    nc.sync.dma_start(
                        dbg_vsb[:, kc_ * 128:(kc_ + 1) * 128],
                        vsb[(0, kc_)][:])
            pb, pq, pctx = pend
            ctsw = emit_norm(pctx)
            for qc in range(4):
                emit_dense(pb, pq, qc, ctsw, pool=ps_sc, tag="sc",
                           engs=(nc.vector, nc.scalar))
    nc.compile()
    return nc


def _prepare_inputs(hidden_states, qkv_w, qkv_b, dense_w):
    """Per-core host-side slicing/transposition/rounding."""
    x = np.ascontiguousarray(hidden_states, dtype=np.float32).reshape(BS, H)
    xt = _bf16(x.T)
    qkv_w = np.asarray(qkv_w, dtype=np.float32)
    qkv_b = np.asarray(qkv_b, dtype=np.float32)
    dense_w = np.asarray(dense_w, dtype=np.float32)
    in_maps = []
    for c in range(NCORES):
        h0, h1 = 2 * c, 2 * c + 1
        perm_qk = np.r_[h0 * 192:h0 * 192 + 64, h1 * 192:h1 * 192 + 64,
                        h0 * 192 + 64:h0 * 192 + 128,
                        h1 * 192 + 64:h1 * 192 + 128]
        perm_v = np.r_[h0 * 192 + 128:h0 * 192 + 192,
                       h1 * 192 + 128:h1 * 192 + 192]
        w1qk = _bf16(qkv_w[perm_qk, :].T)            # [1024, 256]
        w1v_t = qkv_w[perm_v, :].T                   # [1024, 128]
        # crest1[p, k*128 + m] = w1v_t[k*128 + p, m]
        crest1 = _bf16(np.ascontiguousarray(
            w1v_t.reshape(NK, 128, 128).transpose(1, 0, 2).reshape(128, 1024)))
        qkb = np.ascontiguousarray(
            np.stack([qkv_b[perm_qk[0:128]], qkv_b[perm_qk[128:256]]],
                     axis=1), dtype=np.float32)      # [128, 2]
        w2c = dense_w[:, c * 128:(c + 1) * 128].T    # [128, 1024]
        crest2 = _bf16(np.concatenate(
            [np.eye(128, dtype=np.float32), w2c], axis=1))  # [128, 1152]
        in_maps.append({
            "xt": xt, "w1qk": w1qk, "crest1": crest1, "crest2": crest2,
            "qkb": qkb,
        })
    return in_maps


def _reference_numpy(hidden_states, attention_mask, qkv_w, qkv_b, dense_w,
                     dense_b):
    """Exact fallback for non-all-ones masks (never hit with spec inputs)."""
    x = np.asarray(hidden_states, dtype=np.float64)
    mask = np.asarray(attention_mask, dtype=np.float64)
    mixed = x @ np.asarray(qkv_w, np.float64).T + np.asarray(qkv_b, np.float64)
    mixed = mixed.reshape(B, S, NH, 3 * HD).transpose(0, 2, 1, 3)
    q, k, v = np.split(mixed, 3, axis=-1)
    scores = np.einsum("bhqd,bhkd->bhqk", q, k) / np.sqrt(HD)
    scores = scores * mask - 10000.0 * (1.0 - mask)
    scores -= scores.max(axis=-1, keepdims=True)
    probs = np.exp(scores)
    probs /= probs.sum(axis=-1, keepdims=True)
    cx = np.einsum("bhqk,bhkd->bhqd", probs, v)
    cx = cx.transpose(0, 2, 1, 3).reshape(B, S, H)
    o = cx @ np.asarray(dense_w, np.float64).T + np.asarray(dense_b, np.float64)
    return o.astype(np.float32)


def _run(inputs, trace=False):
    from concourse.bass_utils import run_bass_kernel_spmd
    if "nc" not in _CACHE:
        _CACHE["nc"] = _build_program()
    nc = _CACHE["nc"]
    in_maps = _prepare_inputs(inputs["hidden_states"], inputs["qkv_w"],
                              inputs["qkv_b"], inputs["dense_w"])
    res = run_bass_kernel_spmd(nc, in_maps, core_ids=list(range(NCORES)),
                               trace=trace)
    partials = np.stack([np.asarray(r["out"], dtype=np.float64)
                         for r in res.results], axis=0)
    full = partials.sum(axis=0)
    qkv_b = np.asarray(inputs["qkv_b"], dtype=np.float64)
    dense_w = np.asarray(inputs["dense_w"], dtype=np.float64)
    # v-bias folding: ctx = sum_k p_k (v_k + b_v) = sum_k p_k v_k + b_v
    b_v = np.concatenate([qkv_b[h * 192 + 128:h * 192 + 192]
                          for h in range(NH)])
    full += np.asarray(inputs["dense_b"], dtype=np.float64) + dense_w @ b_v
    return full.astype(np.float32).reshape(B, S, H), res


def kernel(hidden_states, attention_mask, qkv_w, qkv_b, dense_w, dense_b):
    hidden_states = np.asarray(hidden_states)
    attention_mask = np.asarray(attention_mask)
    qkv_w = np.asarray(qkv_w)
    qkv_b = np.asarray(qkv_b)
    dense_w = np.asarray(dense_w)
    dense_b = np.asarray(dense_b)
    if not np.all(attention_mask == 1.0):
        return _reference_numpy(hidden_states, attention_mask, qkv_w, qkv_b,
                                dense_w, dense_b)
    out, _ = _run({
        "hidden_states": hidden_states, "qkv_w": qkv_w, "qkv_b": qkv_b,
        "dense_w": dense_w, "dense_b": dense_b,
    }, trace=bool(int(os.environ.get("KERNEL_TRACE", "0"))))
    return out


# revision 53
# speedup vs baseline: 1.1859x; 1.0095x over previous
"""Trainium2 Bass kernel for nn_Attention: 16-head attention, B=2, S=2048, H=1024.

Strategy (Megatron tensor-parallel over heads, 8 cores x 2 heads), v2:
  - All device data in bf16 (fp32 PSUM accumulation), halving HBM traffic.
  - Transposed-context formulation: probabilities are the *stationary* matmul
    operand and V the moving one, so each context matmul streams only 65
    columns (64 dims + a ones column for the softmax denominator) instead of
    512 queries. Stationary loads are free on the PE, halving context cost.
  - V is produced directly in [token, dim] layout by per-token-chunk matmuls
    (x chunk stationary), eliminating the separate V transposes.
  - Normalization is a per-partition scalar multiply (denominator lands in
    the same partition as its query), then a single PE transpose per 128
    tokens feeds the dense projection.
  - The kc loop is software-pipelined (scores run one chunk ahead of the
    exp->context consumers) so the Activation engine never idles; psum->sbuf
    staging runs on DVE/Pool, keeping Act exclusively on exp.
  - Host sums the 8 partial dense outputs (Megatron all-reduce-after-dense),
    adds dense_b and the folded V-bias term.
"""
import os
import numpy as np
import ml_dtypes

B, S, H, NH = 2, 2048, 1024, 16
HD = H // NH            # 64
BS = B * S              # 4096
NCORES = 8
NK = H // 128           # 8 contraction chunks
NQB = S // 512          # 4 query windows per batch
NKC = S // 128          # 16 key chunks per batch

_CACHE = {}
_PHASE_LOG = []   # (label, next-instruction-id) markers, for profiling


def _bf16(x):
    return np.ascontiguousarray(x, dtype=np.float32).astype(ml_dtypes.bfloat16)


def _build_program():
    import concourse.mybir as mybir
    import concourse.tile as tile
    from concourse import bacc
    from contextlib import ExitStack

    F32 = mybir.dt.float32
    BF16 = mybir.dt.bfloat16
    Act = mybir.ActivationFunctionType

    nc = bacc.Bacc("TRN2", target_bir_lowering=False, debug=False,
                   num_devices=NCORES)
    xt = nc.dram_tensor("xt", [H, BS], BF16, kind="ExternalInput").ap()
    w1qk = nc.dram_tensor("w1qk", [H, 256], BF16, kind="ExternalInput").ap()
    # packed consts: crest1 = w1v (8x128 chunks); crest2 = eye | w2t
    crest1 = nc.dram_tensor("crest1", [128, 1024], BF16,
                            kind="ExternalInput").ap()
    crest2 = nc.dram_tensor("crest2", [128, 1152], BF16,
                            kind="ExternalInput").ap()
    qkb = nc.dram_tensor("qkb", [128, 2], F32, kind="ExternalInput").ap()
    out = nc.dram_tensor("out", [BS, H], BF16, kind="ExternalOutput").ap()
    debug = bool(int(os.environ.get("KERNEL_DEBUG", "0")))
    if debug:
        dbg_qt = nc.dram_tensor("dbg_qt", [128, BS], BF16,
                                kind="ExternalOutput").ap()
        dbg_kt = nc.dram_tensor("dbg_kt", [128, BS], BF16,
                                kind="ExternalOutput").ap()
        dbg_vsb = nc.dram_tensor("dbg_vsb", [128, 2 * 2 * NKC * 65], BF16,
                                 kind="ExternalOutput").ap()
        dbg_et = nc.dram_tensor("dbg_et", [128, 1024], BF16,
                                kind="ExternalOutput").ap()
        dbg_cxs = nc.dram_tensor("dbg_cxs", [128, 1024], F32,
                                 kind="ExternalOutput").ap()
        dbg_cts = nc.dram_tensor("dbg_cts", [128, 512], BF16,
                                 kind="ExternalOutput").ap()

    def _mark(label):
        _PHASE_LOG.append(
            (label, int(nc.get_next_instruction_name().split("-")[1])))

    with tile.TileContext(nc) as tc, nc.allow_low_precision(reason="bf16"):
        with ExitStack() as ctx:
            consts = ctx.enter_context(tc.tile_pool(name="consts", bufs=1))
            qkp = ctx.enter_context(tc.tile_pool(name="qkp", bufs=1))
            xtp = ctx.enter_context(tc.tile_pool(name="xtp", bufs=6))
            vsbp = ctx.enter_context(tc.tile_pool(name="vsbp", bufs=34))
            expp = ctx.enter_context(tc.tile_pool(name="expp", bufs=10))
            rsbp = ctx.enter_context(tc.tile_pool(name="rsbp", bufs=2))
            cxp = ctx.enter_context(tc.tile_pool(name="cxp", bufs=3))
            cnp = ctx.enter_context(tc.tile_pool(name="cnp", bufs=3))
            ctsp = ctx.enter_context(tc.tile_pool(name="ctsp", bufs=6))
            obp = ctx.enter_context(tc.tile_pool(name="obp", bufs=4))
            ps_sc = ctx.enter_context(tc.tile_pool(name="ps_sc", bufs=2,
                                                   space="PSUM"))
            ps_ctx = ctx.enter_context(tc.tile_pool(name="ps_ctx", bufs=2,
                                                    space="PSUM"))
            ps_ms = ctx.enter_context(tc.tile_pool(name="ps_ms", bufs=2,
                                                   space="PSUM"))

            # ---- constants ----
            w1qk_sb = consts.tile([128, NK, 256], BF16, name="w1qk")
            w1qk_r = w1qk.rearrange("(k p) m -> p k m", p=128)
            nc.sync.dma_start(w1qk_sb[:, 0:NK // 2, :], w1qk_r[:, 0:NK // 2, :])
            nc.sync.dma_start(w1qk_sb[:, NK // 2:NK, :], w1qk_r[:, NK // 2:NK, :])
            qkb_sb = consts.tile([128, 2], F32, name="qkb")
            nc.sync.dma_start(qkb_sb[:], qkb)
            warm = consts.tile([1, 1], F32, name="warm")
            nc.scalar.activation(warm[0:1, 0:1], qkb_sb[0:1, 0:1], Act.Exp)
            crest1_sb = consts.tile([128, 1024], BF16, name="crest1")
            crest2_sb = consts.tile([128, 1152], BF16, name="crest2")
            w1v_sb = crest1_sb[:, 0:1024].rearrange("p (k m) -> p k m", k=NK)
            eye_sb = crest2_sb[:, 0:128]
            w2_sb = crest2_sb[:, 128:1152]

            qt = qkp.tile([128, BS], BF16, name="qt")
            kt = qkp.tile([128, BS], BF16, name="kt")
            ones_sb = consts.tile([128, 1], BF16, name="ones")
            nc.vector.memset(ones_sb[:], 1.0)
            vsb = {}

            # ---- building blocks ----
            # Every PSUM tile's lifetime (alloc -> matmuls -> drain copy) is
            # emitted contiguously so the misc psum ring can never deadlock
            # on out-of-order buffer reuse.
            def emit_xt_dma(n, halves=False):
                _mark(f"xtdma{n}")
                xt_t = xtp.tile([128, NK, 512], BF16, name="xt")
                src = xt[:, n * 512:(n + 1) * 512].rearrange(
                    "(c p) f -> p c f", p=128)
                if halves:   # split by token half, matching the qk units
                    nc.sync.dma_start(xt_t[:, :, 0:256], src[:, :, 0:256])
                    nc.sync.dma_start(xt_t[:, :, 256:512], src[:, :, 256:512])
                else:
                    nc.sync.dma_start(xt_t[:], src)
                return xt_t

            def _ms_tile(shape, rr=False):
                return ps_ms.tile(shape, F32, name="psqk", tag="misc")

            def emit_qk_unit(n, m, xt_t, rr=False):
                """One projection unit: m=0 -> q rows, m=1 -> k rows.
                Two 256-token halves keep misc psum tiles at 1KB."""
                _mark(f"qk{n}.{'qk'[m]}")
                dst = qt if m == 0 else kt
                for h in (0, 1):
                    ps = _ms_tile([128, 256], rr)
                    for k in range(NK):
                        nc.tensor.matmul(ps[:],
                                         w1qk_sb[:, k, m * 128:(m + 1) * 128],
                                         xt_t[:, k, h * 256:(h + 1) * 256],
                                         start=(k == 0), stop=(k == NK - 1))
                    sl = slice(n * 512 + h * 256, n * 512 + (h + 1) * 256)
                    nc.vector.tensor_scalar_add(dst[:, sl], ps[:],
                                                qkb_sb[:, m:m + 1])

            def alloc_vsb(b):
                pass

            def emit_vT(xt_t, c4, t, rr=False):
                """V (+ones col) for global 128-token chunk t, both heads."""
                _mark(f"vT{t}")
                b, kc = divmod(t, NKC)
                ps = _ms_tile([128, 128], rr)
                for k in range(NK):
                    nc.tensor.matmul(ps[:],
                                     xt_t[:, k, c4 * 128:(c4 + 1) * 128],
                                     w1v_sb[:, k, :],
                                     start=(k == 0), stop=(k == NK - 1))
                vt_sb = vsbp.tile([128, 128], BF16, name="vsb")
                nc.vector.tensor_copy(vt_sb[:], ps[:])
                vsb[(b, kc)] = vt_sb

            def emit_scores_exp(b, qb, kc):
                _mark(f"se.{b}{qb}.{kc}")
                sp = ps_sc.tile([128, 1024], F32, name="sc", tag="sc")
                for j in (0, 1):
                    nc.tensor.matmul(
                        sp[:, j * 512:(j + 1) * 512],
                        kt[64 * j:64 * j + 64,
                           b * S + kc * 128:b * S + (kc + 1) * 128],
                        qt[64 * j:64 * j + 64,
                           b * S + qb * 512:b * S + (qb + 1) * 512],
                        start=True, stop=True)
                et = expp.tile([128, 1024], BF16, name="exp")
                nc.scalar.activation(et[:], sp[:], Act.Exp, scale=0.125)
                return et

            def emit_ctx(b, kc, et, ctxps):
                # start zeroes the whole psum bank, so only the first matmul
                # into each tile starts and only the last stops — the four
                # 65-col query groups all live in that one bank-group.
                _mark(f"cx.{b}.{kc}")
                for j in (0, 1):
                    for qc in range(4):
                        st = et[:, j * 512 + qc * 128:j * 512 + (qc + 1) * 128]
                        nc.tensor.matmul(
                            ctxps[j][:, qc * 128:qc * 128 + 64],
                            st,
                            vsb[(b, kc)][:, j * 64:(j + 1) * 64],
                            start=(kc == 0 and qc == 0),
                            stop=False, skip_group_check=True)
                        nc.tensor.matmul(
                            ctxps[j][:, qc * 128 + 64:qc * 128 + 65],
                            st, ones_sb[:],
                            start=False,
                            stop=(kc == NKC - 1 and qc == 3),
                            skip_group_check=True)

            def emit_norm(ctxps, dbg=False):
                """Drain ctx psum to sbuf fast (frees the accumulation ring
                for the next window), then normalize by the denominator
                column and transpose to [dims, tokens] off the critical
                path; returns the cts tile for the dense stage."""
                _mark("norm")
                cxs = {}
                for j in (0, 1):
                    cxs[j] = cxp.tile([128, 512], F32, name="cxs")
                    nc.vector.tensor_copy(cxs[j][:], ctxps[j][:])
                if dbg:
                    for j in (0, 1):
                        nc.sync.dma_start(dbg_cxs[:, j * 512:(j + 1) * 512],
                                          cxs[j][:])
                rt = rsbp.tile([128, 8, 1], F32, name="recip")
                for j in (0, 1):
                    dview = cxs[j][:].rearrange(
                        "p (g w) -> p g w", w=128)[:, :, 64:65]
                    nc.vector.reciprocal(rt[:, j * 4:(j + 1) * 4, :], dview)
                ctsw = ctsp.tile([128, 512], BF16, name="cts")
                for qc in range(4):
                    cn = cnp.tile([128, 128], BF16, name="cn")
                    for j in (0, 1):
                        nc.vector.tensor_scalar_mul(
                            cn[:, j * 64:(j + 1) * 64],
                            cxs[j][:, qc * 128:qc * 128 + 64],
                            rt[:, j * 4 + qc:j * 4 + qc + 1, 0])
                    pt = ps_ms.tile([128, 128], BF16, name="ctT", tag="misc")
                    nc.tensor.transpose(pt[:], cn[:], eye_sb)
                    nc.vector.tensor_copy(ctsw[:, qc * 128:(qc + 1) * 128],
                                          pt[:])
                if dbg:
                    nc.sync.dma_start(dbg_cts[:], ctsw[:])
                return ctsw

            def emit_dense(b, qb, qc, ctsw, pool=None, tag="misc",
                           split_dma=False, engs=None):
                _mark(f"dn.{b}{qb}.{qc}")
                pool = pool or ps_ms
                ob = obp.tile([128, H], BF16, name="ob")
                row0 = b * S + (qb * 4 + qc) * 128
                for nb in (0, 1):
                    dp = pool.tile([128, 512], F32, name="dp", tag=tag)
                    nc.tensor.matmul(dp[:], ctsw[:, qc * 128:(qc + 1) * 128],
                                     w2_sb[:, nb * 512:(nb + 1) * 512],
                                     start=True, stop=True)
                    sl = slice(nb * 512, (nb + 1) * 512)
                    if engs is not None and engs[(2 * qc + nb) % len(engs)] \
                            is nc.scalar:
                        nc.scalar.copy(ob[:, sl], dp[:])
                    else:
                        nc.vector.tensor_copy(ob[:, sl], dp[:])
                    if split_dma:
                        nc.sync.dma_start(
                            out[row0:row0 + 128, nb * 512:(nb + 1) * 512],
                            ob[:, nb * 512:(nb + 1) * 512])
                if not split_dma:
                    nc.sync.dma_start(out[row0:row0 + 128, :], ob[:])

            # ---- emission schedule ----
            # Per-window kc loop is pipelined one chunk ahead: slot kc emits
            # scores/exp(kc) then ctx(kc-1), so the PE always has the next
            # scores ready before Act finishes the current exp. qkv blocks
            # and vT chunks ride the PE slack inside the windows; the
            # previous window's norm runs at slot 0 and its dense chunks at
            # slots 1/5/9/13.
            # Prologue: only block-0 projection precedes window 0 —
            # everything else rides window slots so the scores/exp chain
            # (which paces the whole kernel) starts as early as possible.
            xts = {}
            xts[0] = emit_xt_dma(0, halves=True)
            nc.sync.dma_start(crest1_sb[:], crest1)
            xts[1] = emit_xt_dma(1)
            emit_qk_unit(0, 1, xts[0])   # k first: scores gate on kt
            emit_qk_unit(0, 0, xts[0])

            windows = [(b, qb) for b in range(B) for qb in range(NQB)]
            # window -> {slot: [hook, ...]}; hooks: ("dma", n) xt load,
            # ("crest2", 0), ("k"/"q", n) projection unit, ("vt", t) chunk.
            def _vt(t):
                return ("vt", t)

            hooks = {
                0: {0: [_vt(0)], 1: [("dma", 2), ("k", 1), _vt(1)],
                    2: [("q", 1), _vt(2)], 3: [_vt(3)], 4: [_vt(4)],
                    5: [("dma", 3), ("crest2", 0), _vt(5)],
                    6: [("k", 2), _vt(6)], 7: [_vt(7)],
                    8: [("q", 2), _vt(8)], 9: [_vt(9)],
                    10: [("k", 3), _vt(10)], 11: [_vt(11)],
                    12: [("q", 3), _vt(12)], 13: [_vt(13)], 14: [_vt(14)],
                    15: [_vt(15)]},
                1: {1: [("dma", 4)], 2: [_vt(16)], 4: [("k", 4)],
                    6: [_vt(17)], 8: [_vt(18)], 10: [_vt(19)]},
                2: {1: [("dma", 5)], 3: [_vt(20)], 4: [("k", 5)],
                    6: [_vt(21)], 8: [("q", 4)], 10: [_vt(22)],
                    12: [_vt(23)]},
                3: {1: [("dma", 6)], 3: [_vt(24)], 4: [("k", 6)],
                    6: [_vt(25)], 8: [_vt(26)], 10: [_vt(27)]},
                4: {1: [("dma", 7)], 3: [("k", 7)], 5: [_vt(28)],
                    6: [("q", 5)], 7: [_vt(29)], 9: [_vt(30)],
                    11: [_vt(31)]},
                5: {4: [("q", 6)]},
                6: {4: [("q", 7)]},
            }

            pend = None        # (b, qb, ctxps) awaiting norm
            cts_store = {}     # window -> (b, qb, ctsw) awaiting dense
            # window -> source windows whose dense chunks it emits
            dense_plan = {1: [0], 2: [], 3: [2], 4: [],
                          5: [1, 4], 6: [3, 5], 7: [6]}
            for w, (b, qb) in enumerate(windows):
                ctxps = {j: ps_ctx.tile([128, 512], F32,
                                        name=f"ctx{j}", tag="acc")
                         for j in (0, 1)}
                dq = []
                for i, srcw in enumerate(dense_plan.get(w, [])):
                    for qc in range(4):
                        dq.append((srcw, qc, 4 * i + qc))
                dslots = {2: 0, 6: 1, 10: 2, 14: 3,
                          4: 4, 8: 5, 12: 6, 15: 7}
                et_prev = None
                for kc in range(NKC):
                    et = emit_scores_exp(b, qb, kc)
                    if debug and w == 0 and kc == 0:
                        nc.sync.dma_start(dbg_et[:], et[:])
                    if kc == 0 and pend is not None:
                        pb, pq, pctx = pend
                        ctsw = emit_norm(pctx, dbg=(debug and w == 1))
                        cts_store[w - 1] = (pb, pq, ctsw)
                        pend = None
                    if kc > 0:
                        emit_ctx(b, kc - 1, et_prev, ctxps)
                    if kc in dslots:
                        for srcw, qc, idx in dq:
                            if idx == dslots[kc]:
                                sb_, sq_, sc_ = cts_store[srcw]
                                emit_dense(sb_, sq_, qc, sc_)
                    for hk in hooks.get(w, {}).get(kc, []):
                        what, n = hk
                        if what == "dma":
                            xts[n] = emit_xt_dma(n)
                        elif what == "crest2":
                            nc.sync.dma_start(crest2_sb[:], crest2)
                        elif what == "k":
                            emit_qk_unit(n, 1, xts[n])
                        elif what == "q":
                            emit_qk_unit(n, 0, xts[n])
                        else:
                            emit_vT(xts[n // 4], n % 4, n)
                    et_prev = et
                emit_ctx(b, NKC - 1, et_prev, ctxps)
                pend = (b, qb, ctxps)
            # emit remaining deferred dense (none expected) then the
            # final window's epilogue
            if debug:
                nc.sync.dma_start(dbg_qt[:], qt[:])
                nc.sync.dma_start(dbg_kt[:], kt[:])
                for kc_ in range(NKC):
                    nc.sync.dma_start(
                        dbg_vsb[:, kc_ * 128:(kc_ + 1) * 128],
                        vsb[(0, kc_)][:])
            # Final epilogue: split the norm across DVE and the now-idle
            # Act engine; dense cycles through both free psum pools.
            pb, pq, pctx = pend
            _mark("fin")
            cxs = {}
            for j in (0, 1):
                cxs[j] = cxp.tile([128, 512], F32, name="cxs")
                if j == 0:
                    nc.vector.tensor_copy(cxs[j][:], pctx[j][:])
                else:
                    nc.scalar.copy(cxs[j][:], pctx[j][:])
            rt = rsbp.tile([128, 8, 1], F32, name="recip")
            for j in (0, 1):
                dview = cxs[j][:].rearrange(
                    "p (g w) -> p g w", w=128)[:, :, 64:65]
                nc.vector.reciprocal(rt[:, j * 4:(j + 1) * 4, :], dview)
            ctsw = ctsp.tile([128, 512], BF16, name="cts")
            for qc in range(4):
                cn = cnp.tile([128, 128], BF16, name="cn")
                for j in (0, 1):
                    sc_ap = rt[:, j * 4 + qc:j * 4 + qc + 1, 0]
                    if j == 0:
                        nc.vector.tensor_scalar_mul(
                            cn[:, 0:64], cxs[0][:, qc * 128:qc * 128 + 64],
                            sc_ap)
                    else:
                        nc.scalar.mul(
                            cn[:, 64:128], cxs[1][:, qc * 128:qc * 128 + 64],
                            sc_ap)
                pt = ps_ms.tile([128, 128], BF16, name="ctT", tag="misc")
                nc.tensor.transpose(pt[:], cn[:], eye_sb)
                nc.vector.tensor_copy(ctsw[:, qc * 128:(qc + 1) * 128],
                                      pt[:])
            fin_pools = ((ps_sc, "sc"), (ps_sc, "sc"),
                         (ps_ctx, "acc"), (ps_ctx, "acc"))
            for qc in range(4):
                p_, t_ = fin_pools[qc % 4]
                emit_dense(pb, pq, qc, ctsw, pool=p_, tag=t_,
                           engs=(nc.vector, nc.scalar))
    nc.compile()
    return nc


def _prepare_inputs(hidden_states, qkv_w, qkv_b, dense_w):
    """Per-core host-side slicing/transposition/rounding."""
    x = np.ascontiguousarray(hidden_states, dtype=np.float32).reshape(BS, H)
    xt = _bf16(x.T)
    qkv_w = np.asarray(qkv_w, dtype=np.float32)
    qkv_b = np.asarray(qkv_b, dtype=np.float32)
    dense_w = np.asarray(dense_w, dtype=np.float32)
    in_maps = []
    for c in range(NCORES):
        h0, h1 = 2 * c, 2 * c + 1
        perm_qk = np.r_[h0 * 192:h0 * 192 + 64, h1 * 192:h1 * 192 + 64,
                        h0 * 192 + 64:h0 * 192 + 128,
                        h1 * 192 + 64:h1 * 192 + 128]
        perm_v = np.r_[h0 * 192 + 128:h0 * 192 + 192,
                       h1 * 192 + 128:h1 * 192 + 192]
        w1qk = _bf16(qkv_w[perm_qk, :].T)            # [1024, 256]
        w1v_t = qkv_w[perm_v, :].T                   # [1024, 128]
        # crest1[p, k*128 + m] = w1v_t[k*128 + p, m]
        crest1 = _bf16(np.ascontiguousarray(
            w1v_t.reshape(NK, 128, 128).transpose(1, 0, 2).reshape(128, 1024)))
        qkb = np.ascontiguousarray(
            np.stack([qkv_b[perm_qk[0:128]], qkv_b[perm_qk[128:256]]],
                     axis=1), dtype=np.float32)      # [128, 2]
        w2c = dense_w[:, c * 128:(c + 1) * 128].T    # [128, 1024]
        crest2 = _bf16(np.concatenate(
            [np.eye(128, dtype=np.float32), w2c], axis=1))  # [128, 1152]
        in_maps.append({
            "xt": xt, "w1qk": w1qk, "crest1": crest1, "crest2": crest2,
            "qkb": qkb,
        })
    return in_maps


def _reference_numpy(hidden_states, attention_mask, qkv_w, qkv_b, dense_w,
                     dense_b):
    """Exact fallback for non-all-ones masks (never hit with spec inputs)."""
    x = np.asarray(hidden_states, dtype=np.float64)
    mask = np.asarray(attention_mask, dtype=np.float64)
    mixed = x @ np.asarray(qkv_w, np.float64).T + np.asarray(qkv_b, np.float64)
    mixed = mixed.reshape(B, S, NH, 3 * HD).transpose(0, 2, 1, 3)
    q, k, v = np.split(mixed, 3, axis=-1)
    scores = np.einsum("bhqd,bhkd->bhqk", q, k) / np.sqrt(HD)
    scores = scores * mask - 10000.0 * (1.0 - mask)
    scores -= scores.max(axis=-1, keepdims=True)
    probs = np.exp(scores)
    probs /= probs.sum(axis=-1, keepdims=True)
    cx = np.einsum("bhqk,bhkd->bhqd", probs, v)
    cx = cx.transpose(0, 2, 1, 3).reshape(B, S, H)
    o = cx @ np.asarray(dense_w, np.float64).T + np.asarray(dense_b, np.float64)
    return o.astype(np.float32)


def _run(inputs, trace=False):
    from concourse.bass_utils import run_bass_kernel_spmd
    if "nc" not in _CACHE:
        _CACHE["nc"] = _build_program()
    nc = _CACHE["nc"]
    in_maps = _prepare_inputs(inputs["hidden_states"], inputs["qkv_w"],
                              inputs["qkv_b"], inputs["dense_w"])
    res = run_bass_kernel_spmd(nc, in_maps, core_ids=list(range(NCORES)),
                               trace=trace)
    partials = np.stack([np.asarray(r["out"], dtype=np.float64)
                         for r in res.results], axis=0)
    full = partials.sum(axis=0)
    qkv_b = np.asarray(inputs["qkv_b"], dtype=np.float64)
    dense_w = np.asarray(inputs["dense_w"], dtype=np.float64)
    # v-bias folding: ctx = sum_k p_k (v_k + b_v) = sum_k p_k v_k + b_v
    b_v = np.concatenate([qkv_b[h * 192 + 128:h * 192 + 192]
                          for h in range(NH)])
    full += np.asarray(inputs["dense_b"], dtype=np.float64) + dense_w @ b_v
    return full.astype(np.float32).reshape(B, S, H), res


def kernel(hidden_states, attention_mask, qkv_w, qkv_b, dense_w, dense_b):
    hidden_states = np.asarray(hidden_states)
    attention_mask = np.asarray(attention_mask)
    qkv_w = np.asarray(qkv_w)
    qkv_b = np.asarray(qkv_b)
    dense_w = np.asarray(dense_w)
    dense_b = np.asarray(dense_b)
    if not np.all(attention_mask == 1.0):
        return _reference_numpy(hidden_states, attention_mask, qkv_w, qkv_b,
                                dense_w, dense_b)
    out, _ = _run({
        "hidden_states": hidden_states, "qkv_w": qkv_w, "qkv_b": qkv_b,
        "dense_w": dense_w, "dense_b": dense_b,
    }, trace=bool(int(os.environ.get("KERNEL_TRACE", "0"))))
    return out
